# revision 71
# baseline (speedup 1.0000x reference)
"""Trainium2 Bass kernel for CSSM (Mamba-style 2D selective scan block).

Sharding: 8 cores = 4 batch x 2 d_inner-halves. Each core computes the
front-end for its batch element, the selective scan for its 96 d_inner
channels x 16 states, and a partial output projection. The host sums the
two partial outputs per batch element.

Key structure (v3):
- proj (1x1) folded into the 3x3 dconv on the host (Wc = dconv @ proj);
  x is DMA'd straight into the padded conv buffer.
- in_proj (xin halves) folded into the causal conv1d on the host; only the
  z third of in_proj remains.
- Wrapped-128 state packing: the 16 states x 96 channels = 1536 scan rows
  are processed as 12 tiles of 128 rows (tile k = state k on rows 0..95,
  plus channels 32j..32j+31 of state 12+k//3 on rows 96..127, j = k%3).
  This cuts per-state scan/mul/exp instruction count 16 -> 12. The
  delta/du tails for j=0 come free from host-extended weights; j=1,2 are
  built with one aligned DVE copy + one partition-shifted SBUF DMA.
- All tensor_tensor_scans on DVE (the scan opcode is illegal on Pool);
  a tunable subset of dBu products runs on gpsimd.
- y accumulation over states via 0/1 "unwrap" matmuls on PE, which also
  fold in D*u via an extended diag(D).
- Custom activation-table-load pass (farthest-next-use) replaces the
  builtin greedy one.
"""
import sys

sys.path.insert(0, "/opt/trn_rl_repo")

import numpy as np

C = 96            # d_model; also channels per d_inner half
DI = 192          # d_inner
NST = 16          # d_state
NTIL = 12         # wrapped-128 scan tiles (16*96 rows / 128)
DTR = 6           # dt_rank
HH = 64
WW = 64
L = HH * WW       # 4096
T = 512           # matmul moving-dim chunk
T2 = 1024         # scan-phase chunk (2 matmul chunks)
PW = WW + 2       # 66: padded row width for the 3x3 conv
G = 68            # left guard of the padded conv buffer
PADLEN = G + (HH + 2) * PW + 68
BLOCKS = ((0, 256), (256, 256), (512, 512), (1024, 1024), (2048, 1024),
          (3072, 1024))
DBU_GP = frozenset(range(2, 12))    # dBu products computed on gpsimd

_CACHE = {}


def _emit(tc, nc, mybir, dram):
    from contextlib import ExitStack

    from concourse import bass

    f32 = mybir.dt.float32
    bf16 = mybir.dt.bfloat16
    AF = mybir.ActivationFunctionType
    OP = mybir.AluOpType

    def chunks(bw):
        return [(c0, min(T, bw - c0)) for c0 in range(0, bw, T)]

    def mmacc(out, pairs, start=True, stop=True, ncols=None):
        """Matmul with free-dim split into even <=512-col PSUM-bank groups."""
        n = ncols if ncols is not None else out.shape[-1]
        nchunk = -(-n // T)
        step = -(-n // nchunk)
        bounds = list(range(0, n, step)) + [n]
        for c0, c1 in zip(bounds[:-1], bounds[1:]):
            for i, (lh, rh) in enumerate(pairs):
                nc.tensor.matmul(out[:, c0:c1], lh, rh[:, c0:c1],
                                 start=start and i == 0,
                                 stop=stop and i == len(pairs) - 1)

    with ExitStack() as ctx:
        ec = ctx.enter_context
        consts = ec(tc.tile_pool(name="consts", bufs=1))
        persist = ec(tc.tile_pool(name="persist", bufs=1))
        dpool = ec(tc.tile_pool(name="dpool", bufs=1, space="DRAM"))
        fw = ec(tc.tile_pool(name="fw", bufs=1))
        pxc2 = ec(tc.tile_pool(name="pxc2", bufs=2))
        pxca = ec(tc.tile_pool(name="pxca", bufs=2))
        pxcb = ec(tc.tile_pool(name="pxcb", bufs=2))
        psz = ec(tc.tile_pool(name="psz", bufs=2))
        pxd = ec(tc.tile_pool(name="pxd", bufs=2))
        dl = ec(tc.tile_pool(name="dl", bufs=2))
        pedt = ec(tc.tile_pool(name="pedt", bufs=1))
        pda = ec(tc.tile_pool(name="pda", bufs=6))
        pdbu = ec(tc.tile_pool(name="pdbu", bufs=5))
        ptmp = ec(tc.tile_pool(name="ptmp", bufs=13))
        hp = ec(tc.tile_pool(name="hp", bufs=3))
        bcA = ec(tc.tile_pool(name="bcA", bufs=2))
        bcB = ec(tc.tile_pool(name="bcB", bufs=1))
        tl = ec(tc.tile_pool(name="tl", bufs=2))
        fps = ec(tc.tile_pool(name="fps", bufs=2, space="PSUM"))
        psy = ec(tc.tile_pool(name="psy", bufs=2, space="PSUM"))
        pmix = ec(tc.tile_pool(name="pmix", bufs=2, space="PSUM"))

        def cload(name, shape, dtype=f32, rearr=None, pool=None):
            t = (pool or consts).tile(list(shape), dtype, tag=name)
            src = dram[name]
            if rearr is not None:
                src = src.rearrange(rearr)
            nc.sync.dma_start(t[:], src)
            return t

        wb = consts.tile([128, 2540], bf16, tag="wblob")
        nc.sync.dma_start(wb[:], dram["wblob"])
        fb = consts.tile([128, 15], f32, tag="fblob")
        nc.sync.dma_start(fb[:], dram["fblob"])
        wdt_sb = wb[0:32, 0:128]
        wout_sb = wb[0:C, 128:224]
        ddiag_sb = wb[:, 224:320]
        wacc_sb = wb[:, 320:608].rearrange("p (j m) -> p j m", m=C)
        wc_sb = wb[0:C, 608:1472].rearrange("p (t m) -> p t m", m=C)
        w2a_sb = wb[0:C, 1472:1984].rearrange("p (t m) -> p t m", m=128)
        w2b_sb = wb[0:C, 1984:2368].rearrange("p (t m) -> p t m", m=C)
        winz_sb = wb[0:C, 2368:2464]
        wxpa_sb = wb[:, 2464:2502]
        wxpb_sb = wb[0:C, 2502:2540]
        bdt_sb = fb[:, 0:1]
        b1da_sb = fb[:, 1:2]
        b1db_sb = fb[0:C, 2:3]
        anegw_sb = fb[:, 3:15]

        carry = persist.tile([128, NTIL], f32, tag="carry")
        xdd = dpool.tile([38, L], bf16, tag="xdd")

        xp0 = persist.tile([C, PADLEN], bf16, tag="xp0")
        nc.gpsimd.memset(xp0[:, 0:G + PW + 1], 0.0)
        nc.gpsimd.memset(xp0[:, G + (HH + 1) * PW + 1:PADLEN], 0.0)
        bview = xp0[:, G + PW + 1: G + (HH + 1) * PW + 1]
        nc.gpsimd.memset(
            bview.rearrange("p (r w) -> p r w", w=PW)[:, :, WW:PW], 0.0)

        state = {"xc2_prev": None}

        def _front(s):
            cs, bw = BLOCKS[s]
            ce = cs + bw
            nrow = bw // WW
            # ---- fused proj+3x3 dconv (<=7-row units) -> xc2 (guarded) ----
            units = []
            ro = 0
            while ro < nrow:
                units.append((ro, min(7, nrow - ro)))
                ro += 7
            xc2 = pxc2.tile([C, 3 + T2], bf16, tag="xc2",
                            name=f"xc2_{s}")[:, :3 + bw]
            if s == 0:
                nc.gpsimd.memset(xc2[:, 0:3], 0.0)
            else:
                pw_ = BLOCKS[s - 1][1]
                nc.gpsimd.tensor_copy(xc2[:, 0:3],
                                      state["xc2_prev"][:, pw_:pw_ + 3])
            for u, (ro, rows) in enumerate(units):
                r0 = cs // WW + ro
                cols = rows * PW
                base = G + (r0 + 1) * PW
                psd = fps.tile([128, T2], f32, tag="fps",
                               name=f"dconv_{s}_{u}")[:C]
                pairs = []
                for tap in range(9):
                    dy, dx = tap // 3, tap % 3
                    shift = (dy - 1) * PW + (dx - 1)
                    pairs.append((wc_sb[:, tap, :],
                                  xp0[:, base + shift: base + shift + cols]))
                mmacc(psd[:, :cols], pairs, ncols=cols)
                srcv = psd[:, :cols].rearrange("p (r w) -> p r w", w=PW)[:, :, 1:65]
                dstv = xc2[:, 3 + ro * WW: 3 + (ro + rows) * WW]
                nc.scalar.activation(dstv.rearrange("p (r w) -> p r w", w=WW),
                                     srcv, AF.Copy)
            state["xc2_prev"] = xc2

            # ---- z third of in_proj + fused in_proj/conv1d, silus ----
            # group a is produced at 128 rows (rows 96..127 duplicate
            # channels 0..31) via host-extended weights.
            psa = fps.tile([128, T2], f32, tag="fps", name=f"c1a_{s}")[:, :bw]
            mmacc(psa, [(w2a_sb[:, k, :], xc2[:, k:k + bw]) for k in range(4)])
            psb = fps.tile([128, T2], f32, tag="fps",
                           name=f"c1b_{s}")[:C, :bw]
            mmacc(psb, [(w2b_sb[:, k, :], xc2[:, k:k + bw]) for k in range(4)])
            psz_p = fps.tile([128, T2], f32, tag="fps",
                             name=f"zp_{s}")[:C, :bw]
            mmacc(psz_p, [(winz_sb[:], xc2[:, 3:3 + bw])])
            sz = psz.tile([C, T2], bf16, tag="sz", name=f"sz_{s}")[:, :bw]
            xc_a = pxca.tile([128, T2], bf16, tag="xc_a",
                             name=f"xc_a_{s}")[:, :bw]
            xc_b = pxcb.tile([C, T2], bf16, tag="xc_b", name=f"xc_b_{s}")[:, :bw]
            nc.scalar.activation(xc_a[:], psa[:], AF.Silu, bias=b1da_sb)
            nc.scalar.activation(xc_b[:], psb[:], AF.Silu, bias=b1db_sb)
            nc.scalar.activation(sz[:], psz_p[:], AF.Silu)

            # ---- x_proj -> x_dbl block, staged to DRAM ----
            psx = fps.tile([128, T2], f32, tag="fps",
                           name=f"xp_{s}")[:38, :bw]
            mmacc(psx, [(wxpa_sb[:], xc_a[:]), (wxpb_sb[:], xc_b[:])])
            x_dbl = pxd.tile([38, T2], bf16, tag="x_dbl", name=f"x_dbl_{s}")[:, :bw]
            nc.scalar.activation(x_dbl[:], psx[:], AF.Copy)
            nc.sync.dma_start(xdd[:, cs:ce], x_dbl[:])

            # ---- B/C broadcasts: two mega-tiles of 6 wrapped tiles each.
            # Main rows (0..95) broadcast B/C of states g*6..g*6+5 in one DMA;
            # tail rows (96..127) use the affine (n_t, j) structure of the
            # wrap (k = 3*(n_t-12)+j) for a second strided broadcast DMA.
            bct = []
            for g in range(2):
                t = (bcA if g == 0 else bcB).tile(
                    [128, 6, 2, T2], bf16, tag=f"bc{g}",
                    name=f"bc_{s}_{g}")[:, :, :, :bw]
                for b_ in range(2):
                    row = xdd[DTR + 16 * b_ + 6 * g:
                              DTR + 16 * b_ + 6 * g + 1, cs:ce]
                    srcb = bass.AP(tensor=row.tensor, offset=row.offset,
                                   ap=[[0, C], [L, 6], [1, bw]])
                    nc.sync.dma_start(t[0:C, :, b_], srcb)
                    for m in range(2):
                        n_t = 12 + 2 * g + m
                        rowt = xdd[DTR + 16 * b_ + n_t:
                                   DTR + 16 * b_ + n_t + 1, cs:ce]
                        srct = bass.AP(tensor=rowt.tensor, offset=rowt.offset,
                                       ap=[[0, 32], [0, 3], [1, bw]])
                        nc.sync.dma_start(t[C:128, 3 * m:3 * m + 3, b_], srct)
                bct.append(t)
            bbcc = [bct[k // 6][:, k % 6] for k in range(NTIL)]

            # ---- delta / du at 128 rows, plus the j=1,2 tail variants ----
            edt = pedt.tile([128, T2], bf16, tag="edt", name=f"edt_{s}")[:, :bw]
            for hh, (c0, cw) in enumerate(chunks(bw)):
                psD = pmix.tile([128, T], f32, tag="pmix",
                                name=f"psD_{s}_{hh}")[:, :cw]
                nc.tensor.matmul(psD[:], wdt_sb[:], x_dbl[:32, c0:c0 + cw])
                nc.scalar.activation(edt[:, c0:c0 + cw], psD[:],
                                     AF.Exp, bias=bdt_sb)
            dpu = [dl.tile([128, 2, T2], bf16, tag=f"dpu{j}",
                           name=f"dpu_{s}_{j}")[:, :, :bw] for j in range(3)]
            dP = [t[:, 0] for t in dpu]
            uP = [t[:, 1] for t in dpu]
            nc.scalar.activation(dP[0][:], edt[:], AF.Ln, bias=1.0)
            nc.vector.tensor_mul(uP[0][:], dP[0][:], xc_a[:])
            for j, ch0 in ((1, 32), (2, 64)):
                nc.vector.tensor_copy(dpu[j][0:C], dpu[0][0:C])
                nc.sync.dma_start(dpu[j][C:128], dpu[0][ch0:ch0 + 32])

            state[("xca", s)] = xc_a
            state[("sz", s)] = sz
            state[("bbcc", s)] = bbcc
            state[("dP", s)] = dP
            state[("uP", s)] = uP

        def _scan(s):
            cs, bw = BLOCKS[s]
            cks = chunks(bw)
            xc_a = state.pop(("xca", s))
            sz = state.pop(("sz", s))
            bbcc = state.pop(("bbcc", s))
            dP = state.pop(("dP", s))
            uP = state.pop(("uP", s))

            tmps = []
            for k in range(NTIL):
                j = k % 3
                dA = pda.tile([128, T2], f32, tag="dA",
                              name=f"dA_{s}_{k}")[:, :bw]
                nc.scalar.activation(dA[:], dP[j][:], AF.Exp,
                                     scale=anegw_sb[:, k:k + 1])
                dBu = pdbu.tile([128, T2], bf16, tag="dBu",
                                name=f"dBu_{s}_{k}")[:, :bw]
                meng = nc.gpsimd if k in DBU_GP else nc.vector
                meng.tensor_mul(dBu[:], uP[j][:], bbcc[k][:, 0, :])

                h = hp.tile([128, T2], bf16, tag="h", name=f"h_{s}_{k}")[:, :bw]
                init = 0.0 if s == 0 else carry[:, k:k + 1]
                nc.vector.tensor_tensor_scan(h[:], dA[:], dBu[:], init,
                                             OP.mult, OP.add)
                nc.vector.tensor_copy(carry[:, k:k + 1], h[:, bw - 1:bw])

                tmp = ptmp.tile([128, T2], bf16, tag="tmp",
                                name=f"tmp_{s}_{k}")[:, :bw]
                nc.vector.tensor_mul(tmp[:], h[:], bbcc[k][:, 1, :])
                tmps.append(tmp)

            # ---- per-chunk accumulation over tiles + D*u, gate, out_proj ----
            for hh, (c0, cw) in enumerate(cks):
                sl = slice(c0, c0 + cw)
                yP = psy.tile([C, T], f32, tag="psy",
                              name=f"yP_{s}_{hh}")[:, :cw]
                for k in range(NTIL):
                    nc.tensor.matmul(yP[:], wacc_sb[:, k % 3, :],
                                     tmps[k][:, sl], start=(k == 0),
                                     stop=False)
                nc.tensor.matmul(yP[:], ddiag_sb[:], xc_a[:, sl],
                                 start=False, stop=True)
                y2 = tl.tile([C, T], bf16, tag="y2", name=f"y2_{s}_{hh}")[:, :cw]
                nc.vector.tensor_tensor(y2[:], yP[:], sz[:, sl], OP.mult)
                outP = pmix.tile([C, T], f32, tag="pmix",
                                 name=f"outP_{s}_{hh}")[:, :cw]
                nc.tensor.matmul(outP[:], wout_sb[:], y2[:])
                osb = tl.tile([C, T], f32, tag="osb",
                              name=f"osb_{s}_{hh}")[:, :cw]
                nc.vector.tensor_copy(osb[:], outP[:])
                nc.sync.dma_start(
                    dram["out_part"][:, cs + c0:cs + c0 + cw], osb[:])

        # Software pipeline, depth 2 (+1 for the x DMA halo row): the x DMA
        # runs at block b, the front end at b-1, the scan stage at b-2.
        # split the x load so front(0) (rows 0..4 + halo) can start early
        xv = xp0[:, G + PW + 1: G + (HH + 1) * PW + 1]
        xv = xv.rearrange("p (r w) -> p r w", w=PW)[:, :, 0:WW]
        srcx = dram["x"].rearrange("p (r w) -> p r w", w=WW)
        nc.sync.dma_start(xv[:, 0:8], srcx[:, 0:8])
        nc.sync.dma_start(xv[:, 8:16], srcx[:, 8:16])
        nc.sync.dma_start(xv[:, 16:HH], srcx[:, 16:HH])

        NBK = len(BLOCKS)
        for b in range(NBK + 1):
            if b < NBK:
                _front(b)
            if b >= 1:
                _scan(b - 1)


def _optimal_act_table_loads(self):
    """Replacement for Bacc.insert_act_table_loads: walk the scheduled Act
    instruction stream and insert table loads with farthest-next-use table
    choice (the builtin pass greedily picks the first set containing each
    function, which flaps between the exp-only and ln-only sets)."""
    from concourse import mybir
    from concourse.hw_specs import get_activation_tables

    tables = list(get_activation_tables(self.m.arch).items())
    table_funcs = [funcs for _, funcs in tables]

    for block in self.main_func.blocks:
        has_act = any(isinstance(i, mybir.InstActivation)
                      for i in block.instructions)
        if not has_act:
            continue
        acts = []
        for idx, inst in enumerate(block.instructions):
            if isinstance(inst, mybir.InstActivation):
                cand = frozenset(ti for ti, fs in enumerate(table_funcs)
                                 if inst.func in fs)
                assert cand, f"no act table contains {inst.func}"
                acts.append((idx, cand))
        cur = None
        inserts = []   # (position, set_id)
        for kk, (idx, cand) in enumerate(acts):
            if cur in cand:
                continue
            best, best_len = None, -1
            for ti in cand:
                run = 0
                for _, c2 in acts[kk:]:
                    if ti in c2:
                        run += 1
                    else:
                        break
                if run > best_len:
                    best, best_len = ti, run
            cur = best
            inserts.append((idx, best))
        for pos, set_id in reversed(inserts):
            load = mybir.InstLoadActFuncSet(
                name=f"atlopt_{pos}", ins=[], outs=[], act_func_set_id=set_id)
            load.engine = mybir.EngineType.Activation
            block.instructions.insert(pos, load)


def _build_program():
    from concourse import bacc, tile, mybir

    nc = bacc.Bacc("TRN2", target_bir_lowering=False, debug=False, num_devices=8)
    f32 = mybir.dt.float32
    bf16 = mybir.dt.bfloat16

    def din(name, shape, dtype=f32):
        return nc.dram_tensor(name, shape, dtype, kind="ExternalInput").ap()

    dram = {
        "x": din("x", (C, L), bf16),
        "wblob": din("wblob", (128, 2540), bf16),
        "fblob": din("fblob", (128, 15)),
        "out_part": nc.dram_tensor("out_part", (C, L), f32,
                                   kind="ExternalOutput").ap(),
    }

    with tile.TileContext(nc) as tc:
        _emit(tc, nc, mybir, dram)
    import types

    nc.insert_act_table_loads = types.MethodType(_optimal_act_table_loads, nc)
    nc.compile()
    return nc


def get_program():
    if "nc" not in _CACHE:
        _CACHE["nc"] = _build_program()
    return _CACHE["nc"]


def make_core_inputs(inputs, b, half):
    import ml_dtypes

    bf = ml_dtypes.bfloat16
    perm = np.concatenate([
        np.arange(half * C, half * C + C),
        np.arange((1 - half) * C, (1 - half) * C + C),
    ])
    loc = perm[:C]

    a = np.exp(np.asarray(inputs["A_log"], np.float64))[loc]   # (96, 16)

    # fused proj + dconv: Wc[tap] = (dconv_w[:,:,tap] @ proj_w)^T  (96c, 96o)
    dw = np.asarray(inputs["dconv_w"], np.float64)     # (96, 192, 3, 3)
    pw = np.asarray(inputs["proj_w"], np.float64)[:, :, 0, 0]   # (192, 96)
    wc = np.empty((9, C, C), np.float32)
    for tap in range(9):
        dy, dx = tap // 3, tap % 3
        wc[tap] = (dw[:, :, dy, dx] @ pw).T.astype(np.float32)

    # fused in_proj + causal conv1d:
    #   W2[g][k][c, d] = in_proj_w[perm[g*96+d], c] * conv1d_w[perm[g*96+d], k]
    w_in = np.asarray(inputs["in_proj_w"], np.float64)  # (384, 96)
    w1 = np.asarray(inputs["conv1d_w"], np.float64)     # (192, 4)
    w2 = np.empty((2, 4, C, C), np.float32)
    for g in range(2):
        rows = perm[g * C:(g + 1) * C]
        wg = w_in[rows].T                               # (96c, 96d)
        for k in range(4):
            w2[g, k] = (wg * w1[rows, k][None, :]).astype(np.float32)
    # group a extended to 128 output rows (tail duplicates channels 0..31)
    w2a = np.zeros((4, C, 128), np.float32)
    w2a[:, :, :C] = w2[0]
    w2a[:, :, C:] = w2[0][:, :, :32]
    winz = np.ascontiguousarray(w_in[DI + loc].T).astype(np.float32)

    b1 = np.asarray(inputs["conv1d_b"], np.float32)[perm]
    b1da = np.zeros((128, 1), np.float32)
    b1da[:C, 0] = b1[:C]
    b1da[C:, 0] = b1[:32]
    b1db = b1[C:, None].copy()

    wxp_full = np.asarray(inputs["x_proj_w"], np.float32)[:, perm]  # (38, 192)
    wxpa = np.zeros((128, 38), np.float32)
    wxpa[:C] = wxp_full[:, :C].T
    wxpb = np.ascontiguousarray(wxp_full[:, C:].T)

    wdt_ = np.zeros((32, C), np.float32)
    wdt_[:DTR] = np.asarray(inputs["dt_proj_w"], np.float32)[loc].T
    wdt = np.zeros((32, 128), np.float32)
    wdt[:, :C] = wdt_
    wdt[:, C:] = wdt_[:, :32]

    bdt_ = np.asarray(inputs["dt_proj_b"], np.float32)[loc]
    bdt = np.zeros((128, 1), np.float32)
    bdt[:C, 0] = bdt_
    bdt[C:, 0] = bdt_[:32]

    # wrapped-tile decay rates: col k rows 0..95 -> state k, rows 96..127 ->
    # state 12+k//3 on channels 32*(k%3)..+32
    anegw = np.zeros((128, NTIL), np.float32)
    for k in range(NTIL):
        n_t = 12 + k // 3
        ch0 = 32 * (k % 3)
        anegw[:C, k] = -a[:, k]
        anegw[C:, k] = -a[ch0:ch0 + 32, n_t]

    dvec = np.asarray(inputs["D"], np.float32)[loc]
    ddiag = np.zeros((128, C), np.float32)
    ddiag[:C] = np.diag(dvec)

    # unwrap/accumulate matrices: pattern j maps tail row 96+i -> out 32j+i
    wacc = np.zeros((3, 128, C), np.float32)
    for j in range(3):
        wacc[j, :C] = np.eye(C)
        for i in range(32):
            wacc[j, C + i, 32 * j + i] = 1.0

    wblob = np.zeros((128, 2540), np.float32)
    wblob[0:32, 0:128] = wdt
    wblob[0:C, 128:224] = np.asarray(inputs["out_proj_w"],
                                     np.float32)[:, loc].T
    wblob[:, 224:320] = ddiag
    for j in range(3):
        wblob[:, 320 + C * j:320 + C * (j + 1)] = wacc[j]
    for tap in range(9):
        wblob[0:C, 608 + C * tap:608 + C * (tap + 1)] = wc[tap]
    for t in range(4):
        wblob[0:C, 1472 + 128 * t:1472 + 128 * (t + 1)] = w2a[t]
        wblob[0:C, 1984 + C * t:1984 + C * (t + 1)] = w2[1][t]
    wblob[0:C, 2368:2464] = winz
    wblob[:, 2464:2502] = wxpa
    wblob[0:C, 2502:2540] = wxpb

    fblob = np.zeros((128, 15), np.float32)
    fblob[:, 0:1] = bdt
    fblob[:, 1:2] = b1da
    fblob[0:C, 2:3] = b1db
    fblob[:, 3:15] = anegw

    return {
        "x": np.ascontiguousarray(
            np.asarray(inputs["x"], np.float32)[b].reshape(C, L)).astype(bf),
        "wblob": wblob.astype(bf),
        "fblob": fblob,
    }


def kernel(**inputs):
    from concourse import bass_utils

    nc = get_program()
    in_maps = [make_core_inputs(inputs, b, half)
               for b in range(4) for half in range(2)]
    res = bass_utils.run_bass_kernel_spmd(nc, in_maps, core_ids=list(range(8)))
    out = np.zeros((4, C, L), np.float32)
    for b in range(4):
        out[b] = res.results[2 * b]["out_part"] + res.results[2 * b + 1]["out_part"]
    return out.reshape(4, C, HH, WW)
